# revision 4
# baseline (speedup 1.0000x reference)
"""CoAttention kernel for Trainium2, 8 NeuronCores, data-parallel over batch.

Reference computation (per batch b):
    k_proj = key @ W_k.T + b_k            # (S, D)
    scores = query @ k_proj.T             # (S, S)
    scores += log(cell_mask) + log(seq_mask)[None, :]
    p = softmax(scores, axis=-1)
    out = (p @ value) @ W_o.T + b_o       # (S, D)

Algebraic simplifications used (exact):
  - scores = query @ W_k @ key.T + (query @ b_k)[:, None]; the b_k term is
    constant along the softmax axis, so softmax is invariant to it -> b_k
    is dropped entirely.
  - cell_mask/seq_mask are all-ones per the problem spec (log == 0); the
    kernel checks this on the host and falls back to a numpy path if not.
  - b_o is added on the host (it is all-zeros per spec, but handled exactly).
  - W_o is folded into value upfront: vW = value @ W_o.T (f16), and
    out = (expv @ vW) * recip -- softmax normalization commutes with the
    output projection. This removes the x-transpose stage entirely.

Precision scheme (SCORES_DT = "f32r" default):
  - scores path (q_projT = (query @ W_k)^T, scores = q_projT^T @ kT) in a
    SINGLE f32r (fp32-storage, ~tf32 internal) matmul pass, fp32 PSUM.
    Measured on HW: f32r per-operand error ~1.6e-4 -> output rel err ~6e-3
    against a 2e-2 gate. SCORES_DT="f16" is a slightly cheaper/less
    accurate variant (~1.3e-2).
  - softmax: fp32 row max (negated) on DVE, exp on ScalarE (f16 out) with
    fused fp32 row-sum; reciprocal on DVE.
  - tail (expv @ vW) in f16 with fp32 PSUM, normalized by recip on DVE
    during PSUM evacuation.

Schedule: phase A interleaves W_k/query loads + casts + PE transposes with
the q_proj matmuls; phase C' builds kT and overlaps scores(0); phase B
builds vW = v @ W_o.T while softmax(0) runs on DVE/ACT; the main loop is
software-pipelined as scores(qb) / pT+out(qb-1) with PSUM split exactly
into 8 banks: scores 4, pT transposes 2, out accumulator 2.
"""

import os as _os

import numpy as np

import concourse.bass as bass
import concourse.mybir as mybir
import concourse.tile as tile
from concourse import bacc
from concourse.bass_utils import run_bass_kernel_spmd
from concourse.masks import make_identity

P = 128
S = 2048
D = 1024
NBS = S // P   # 16 row blocks of seq
NBD = D // P   # 8 row blocks of feature dim
NC = 8         # cores == batch
F32 = mybir.dt.float32
F16 = mybir.dt.float16
F32R = mybir.dt.float32r

SCORES_DT = _os.environ.get("KERNEL_SCORES_DT", "f32r")


def build_nc(scores_dt=SCORES_DT, repeat=1):
    nc = bacc.Bacc("TRN2", target_bir_lowering=False, debug=False)
    d_query = nc.dram_tensor("query", [S, D], F32, kind="ExternalInput")
    d_key = nc.dram_tensor("key", [S, D], F32, kind="ExternalInput")
    d_value = nc.dram_tensor("value", [S, D], F32, kind="ExternalInput")
    d_wk = nc.dram_tensor("W_k", [D, D], F32, kind="ExternalInput")
    d_wo = nc.dram_tensor("W_o", [D, D], F32, kind="ExternalInput")
    d_out = nc.dram_tensor("out", [S, D], F32, kind="ExternalOutput")

    s_dt = {"f32r": F32R, "f16": F16}[scores_dt]

    with tile.TileContext(nc) as tc:
      def emit_body():
        # ---------------- constants ----------------
        const_pool = tc.alloc_tile_pool(name="const", bufs=1)
        ident16 = const_pool.tile([P, P], F16)
        make_identity(nc, ident16[:])
        if s_dt == F16:
            id_s = ident16
        else:
            ident32 = const_pool.tile([P, P], F32)
            make_identity(nc, ident32[:])
            id_s = const_pool.tile([P, P], s_dt)
            nc.vector.tensor_copy(id_s[:], ident32[:])

        # one shared rotating pool for ALL hbm loads; DMA issue order ==
        # consumer emission order: wk, query, key, wo, value
        ld = tc.alloc_tile_pool(name="ld", bufs=4)

        def load(dram, i, tag="ld"):
            t = ld.tile([P, D], F32, tag=tag, name=f"ld_{dram.name}_{i}")
            nc.sync.dma_start(t[:], dram[i * P:(i + 1) * P, :])
            return t

        # resident: q_projT [d-part (8 blocks), q] ; block db at cols db*S
        qpT_pool = tc.alloc_tile_pool(name="qpT", bufs=1)
        qpT = qpT_pool.tile([P, NBD * S], s_dt, name="qpT")

        # ============ phase A: wk + query -> q_projT ============
        with tc.tile_pool(name="pa_sb", bufs=1) as pa_sb, \
             tc.tile_pool(name="pa_st", bufs=3) as pa_st, \
             tc.tile_pool(name="pa_tr", bufs=2, space="PSUM") as pa_tr, \
             tc.tile_pool(name="pa_ps", bufs=2, space="PSUM") as pa_ps:

            wk = [pa_sb.tile([P, D], s_dt, name=f"wk{i}") for i in range(NBD)]
            qt = pa_sb.tile([P, NBD * S], s_dt, name="qt")
            qt3 = qt[:].rearrange("p (j s) -> p j s", j=NBD)

            def do_q_tile(qi):
                t = load(d_query, qi)
                stg = pa_st.tile([P, D], s_dt, tag="qstg", name=f"qstg{qi}")
                nc.vector.tensor_copy(stg[:], t[:])
                tr = pa_tr.tile([P, NBD * P], s_dt, tag="tr", name=f"qtr{qi}")
                for j in range(NBD):
                    nc.tensor.transpose(tr[:, j * P:(j + 1) * P],
                                        stg[:, j * P:(j + 1) * P], id_s[:])
                nc.scalar.copy(qt3[:, :, qi * P:(qi + 1) * P],
                               tr[:].rearrange("p (j q) -> p j q", j=NBD))

            def do_qp_chunk(qc):
                for db in range(NBD):
                    ps = pa_ps.tile([P, 512], F32, tag="qp",
                                    name=f"qp{db}_{qc}")
                    for dpb in range(NBD):
                        nc.tensor.matmul(
                            ps[:],
                            wk[dpb][:, db * P:(db + 1) * P],
                            qt[:, dpb * S + qc * 512:dpb * S + (qc + 1) * 512],
                            start=(dpb == 0), stop=(dpb == NBD - 1))
                    nc.vector.tensor_copy(
                        qpT[:, db * S + qc * 512:db * S + (qc + 1) * 512],
                        ps[:])

            for qi in range(4):
                do_q_tile(qi)
            for i in range(NBD):
                t = load(d_wk, i)
                nc.vector.tensor_copy(wk[i][:], t[:])
            for qc in range(4):
                do_qp_chunk(qc)
                if qc < 3:
                    for qi in range(4 * (qc + 1), 4 * (qc + 2)):
                        do_q_tile(qi)

        # resident: vW = value @ W_o.T, f16, [k-part (16 tiles), o]
        vw_pool = tc.alloc_tile_pool(name="vw", bufs=1)
        vw = [vw_pool.tile([P, D], F16, name=f"vw{i}") for i in range(NBS)]

        # ============ phase B: wo + value -> vW ============
        # (before kT so vt/wot never coexist with kT: SBUF peak stays <190KB)
        with tc.tile_pool(name="pb_sb", bufs=1) as pb_sb, \
             tc.tile_pool(name="pb_st", bufs=3) as pb_st, \
             tc.tile_pool(name="pb_tr", bufs=2, space="PSUM") as pb_tr, \
             tc.tile_pool(name="pb_ps", bufs=2, space="PSUM") as pb_ps:

            wot = pb_sb.tile([P, NBD * D], F16, name="wot")
            wot3 = wot[:].rearrange("p (j o) -> p j o", j=NBD)
            vt = pb_sb.tile([P, NBD * S], F16, name="vt")
            vt3 = vt[:].rearrange("p (j s) -> p j s", j=NBD)

            def tr16(dst3, src, i, tag):
                tr = pb_tr.tile([P, NBD * P], F16, tag="tr", name=f"{tag}{i}")
                for j in range(NBD):
                    nc.tensor.transpose(tr[:, j * P:(j + 1) * P],
                                        src[:, j * P:(j + 1) * P], ident16[:])
                nc.scalar.copy(dst3[:, :, i * P:(i + 1) * P],
                               tr[:].rearrange("p (j q) -> p j q", j=NBD))

            for oi in range(NBD):
                t = load(d_wo, oi)
                stg = pb_st.tile([P, D], F16, tag="wstg", name=f"wos{oi}")
                nc.vector.tensor_copy(stg[:], t[:])
                tr16(wot3, stg[:], oi, "wotr")
            for ki in range(NBS):
                t = load(d_value, ki)
                stg = pb_st.tile([P, D], F16, tag="vstg", name=f"vs{ki}")
                nc.vector.tensor_copy(stg[:], t[:])
                tr16(vt3, stg[:], ki, "vtr")
            for kb in range(NBS):
                for oh in range(2):
                    ps = pb_ps.tile([P, 512], F32, tag="vw",
                                    name=f"vwp{kb}_{oh}")
                    for db in range(NBD):
                        nc.tensor.matmul(
                            ps[:],
                            vt[:, db * S + kb * P:db * S + (kb + 1) * P],
                            wot[:, db * D + oh * 512:db * D + (oh + 1) * 512],
                            start=(db == 0), stop=(db == NBD - 1))
                    nc.vector.tensor_copy(vw[kb][:, oh * 512:(oh + 1) * 512],
                                          ps[:])

        # scores PSUM: 4 banks, single buffer, lives until the loop ends
        sc_ps = tc.alloc_tile_pool(name="sc_ps", bufs=1, space="PSUM")

        # resident: keyT [d-part (8 blocks), k]
        kT_pool = tc.alloc_tile_pool(name="kT", bufs=1)
        kT = kT_pool.tile([P, NBD * S], s_dt, name="kT")
        kT3 = kT[:].rearrange("p (j s) -> p j s", j=NBD)

        # softmax-state pools (used from phase C' for q-block 0)
        exp_sb = tc.alloc_tile_pool(name="exp_sb", bufs=2)
        st_sb = tc.alloc_tile_pool(name="st_sb", bufs=2)

        state = {}

        def scores_chunk(qb, kc, scores):
            q0 = qb * P
            for db in range(NBD):
                nc.tensor.matmul(
                    scores[:, kc * 512:(kc + 1) * 512],
                    qpT[:, db * S + q0:db * S + q0 + P],
                    kT[:, db * S + kc * 512:db * S + (kc + 1) * 512],
                    start=(db == 0), stop=(db == NBD - 1))

        def do_negmax(qb):
            st = state[qb]
            neg_max = st_sb.tile([P, 1], F32, tag="negmax", name=f"nm{qb}")
            nc.vector.reduce_max(neg_max[:], st["scores"][:],
                                 axis=mybir.AxisListType.X, negate=True)
            st["neg_max"] = neg_max

        def do_exp(qb):
            st = state[qb]
            rowsum = st_sb.tile([P, 1], F32, tag="rowsum", name=f"rs{qb}")
            expv = exp_sb.tile([P, S], F16, tag="expv", name=f"expv{qb}")
            nc.scalar.activation(expv[:], st["scores"][:],
                                 mybir.ActivationFunctionType.Exp,
                                 bias=st["neg_max"][:], scale=1.0,
                                 accum_out=rowsum[:])
            recip = st_sb.tile([P, 1], F32, tag="recip", name=f"rc{qb}")
            nc.vector.reciprocal(recip[:], rowsum[:])
            st["expv"] = expv
            st["recip"] = recip

        # ============ phase C': key -> kT overlapped with scores(0) ============
        with tc.tile_pool(name="pc_st", bufs=3) as pc_st, \
             tc.tile_pool(name="pc_tr", bufs=2, space="PSUM") as pc_tr:

            scores0 = sc_ps.tile([P, S], F32, tag="scores", name="scores_0")
            state[0] = {"scores": scores0}

            for kc in range(4):
                for ki in range(4 * kc, 4 * (kc + 1)):
                    t = load(d_key, ki)
                    stg = pc_st.tile([P, D], s_dt, tag="kstg", name=f"ks{ki}")
                    nc.vector.tensor_copy(stg[:], t[:])
                    tr = pc_tr.tile([P, NBD * P], s_dt, tag="tr",
                                    name=f"ktr{ki}")
                    for j in range(NBD):
                        nc.tensor.transpose(tr[:, j * P:(j + 1) * P],
                                            stg[:, j * P:(j + 1) * P],
                                            id_s[:])
                    nc.scalar.copy(kT3[:, :, ki * P:(ki + 1) * P],
                                   tr[:].rearrange("p (j q) -> p j q", j=NBD))
                scores_chunk(0, kc, scores0)
            do_negmax(0)
        do_exp(0)

        # ============ main loop over q blocks (software-pipelined) ============
        tr_ps = tc.alloc_tile_pool(name="tr_ps", bufs=2, space="PSUM")
        o_ps = tc.alloc_tile_pool(name="o_ps", bufs=1, space="PSUM")
        pt_sb = tc.alloc_tile_pool(name="pt_sb", bufs=2)
        out_sb = tc.alloc_tile_pool(name="out_sb", bufs=2)

        def pt_build(qb):
            """Transpose expv(qb) -> pT in 2 groups of 8 blocks (1 PSUM bank
            each), evacuated on ACT."""
            st = state[qb]
            expv = st["expv"]
            pts = []
            for g in range(2):
                trp = tr_ps.tile([P, 8 * P], F16, tag="trp",
                                 name=f"ptp{qb}_{g}")
                for j in range(8):
                    kb = g * 8 + j
                    nc.tensor.transpose(trp[:, j * P:(j + 1) * P],
                                        expv[:, kb * P:(kb + 1) * P],
                                        ident16[:])
                pt = pt_sb.tile([P, 8 * P], F16, tag="pt", name=f"pt{qb}_{g}")
                nc.scalar.copy(pt[:], trp[:])
                pts.append(pt)
            st["pts"] = pts

        def out_mm(qb):
            st = state.pop(qb)
            pts, recip = st["pts"], st["recip"]
            op = o_ps.tile([P, D], F32, tag="op", name=f"op{qb}")
            for oh in range(2):
                for kb in range(NBS):
                    nc.tensor.matmul(
                        op[:, oh * 512:(oh + 1) * 512],
                        pts[kb // 8][:, (kb % 8) * P:(kb % 8 + 1) * P],
                        vw[kb][:, oh * 512:(oh + 1) * 512],
                        start=(kb == 0), stop=(kb == NBS - 1))
            osb = out_sb.tile([P, D], F32, tag="osb", name=f"osb{qb}")
            nc.vector.tensor_scalar_mul(osb[:], op[:], recip[:])
            q0 = qb * P
            nc.sync.dma_start(d_out[q0:q0 + P, :], osb[:])

        for qb in range(1, NBS + 1):
            if qb < NBS:
                scores = sc_ps.tile([P, S], F32, tag="scores",
                                    name=f"scores_{qb}")
                state[qb] = {"scores": scores}
                for kc in range(4):
                    scores_chunk(qb, kc, scores)
                do_negmax(qb)
            pt_build(qb - 1)
            if qb < NBS:
                do_exp(qb)
            out_mm(qb - 1)

        out_sb.release()
        pt_sb.release()
        o_ps.release()
        tr_ps.release()
        st_sb.release()
        exp_sb.release()
        kT_pool.release()
        sc_ps.release()
        vw_pool.release()
        qpT_pool.release()
        ld.release()
        const_pool.release()

      for _rep in range(repeat):
          emit_body()

    nc.compile()
    return nc


_NC_CACHE = {}


def _get_nc():
    if "nc" not in _NC_CACHE:
        _NC_CACHE["nc"] = build_nc()
    return _NC_CACHE["nc"]


def _numpy_fallback(query, key, value, cell_mask, seq_mask, W_k, b_k, W_o, b_o):
    out = np.empty((query.shape[0], S, D), dtype=np.float32)
    for b in range(query.shape[0]):
        kp = key[b].astype(np.float64) @ W_k.astype(np.float64).T + b_k
        s = query[b].astype(np.float64) @ kp.T
        s = s + np.log(cell_mask[b]) + np.log(seq_mask[b])[None, :]
        s -= s.max(1, keepdims=True)
        e = np.exp(s)
        p = e / e.sum(1, keepdims=True)
        x = p @ value[b].astype(np.float64)
        out[b] = (x @ W_o.astype(np.float64).T + b_o).astype(np.float32)
    return out


def kernel(query, key, value, cell_mask, seq_mask, W_k, b_k, W_o, b_o):
    query = np.ascontiguousarray(query, dtype=np.float32)
    key = np.ascontiguousarray(key, dtype=np.float32)
    value = np.ascontiguousarray(value, dtype=np.float32)
    W_k = np.ascontiguousarray(W_k, dtype=np.float32)
    W_o = np.ascontiguousarray(W_o, dtype=np.float32)

    # masks are all-ones per the problem spec -> log-mask bias is exactly 0.
    # b_k shifts every score row by a constant -> softmax-invariant (exact).
    if not (np.all(np.asarray(cell_mask) == 1.0)
            and np.all(np.asarray(seq_mask) == 1.0)):
        return _numpy_fallback(np.asarray(query), np.asarray(key),
                               np.asarray(value), np.asarray(cell_mask),
                               np.asarray(seq_mask), W_k,
                               np.asarray(b_k), W_o, np.asarray(b_o))

    nc = _get_nc()
    in_maps = [
        {"query": query[b], "key": key[b], "value": value[b],
         "W_k": W_k, "W_o": W_o}
        for b in range(NC)
    ]
    res = run_bass_kernel_spmd(nc, in_maps, core_ids=list(range(NC)))
    out = np.stack([res.results[b]["out"] for b in range(NC)])
    if b_o is not None and np.any(np.asarray(b_o) != 0.0):
        out = out + np.asarray(b_o, dtype=np.float32)[None, None, :]
    return out
